# revision 16
# baseline (speedup 1.0000x reference)
"""Trainium2 Bass kernel for KV-cache int4 fake-quantization (quantize +
pack + concat + dequantize).

Math (per row of D=128 features):
    scale = max(absmax(x)/7, 1e-8)
    xi    = clip(round(x/scale), -7, 7)      # clip never binds: |x/scale| <= 7
    out   = xi * scale
The int4 pack/unpack round-trips exactly, so it is elided. The seq-dim
concat is pure data placement handled by output DMA offsets.

Sharding: B*H = 64 (batch, head) pairs split 8-way across cores; all work
is row-local so there is no communication.

Implementation notes (hardware-verified):
  - tensor_reduce(apply_absolute_value=True) gives per-row absmax in one
    1x pass.
  - DVE f32->int8 store conversion rounds to nearest-even (matches
    jnp.round) and saturates, so pass1 is a single broadcast multiply
    x * (1/scale) with int8 output: one 1x DVE op per tile.
  - Dequant (pass2) is a broadcast multiply xi * scale, int8 in / f32
    out; split per-tile between DVE (tensor_tensor with a step-0
    broadcast AP), ACT (per-row-slice activation Copy with per-partition
    scale), and GPSIMD (tensor_tensor broadcast) to balance engine time
    under the DMA roofline.
"""

import sys

sys.path.insert(0, "/opt/trn_rl_repo")

import numpy as np

import concourse.bass as bass
import concourse.tile as tile
from concourse import bacc, mybir
from concourse.bass_utils import run_bass_kernel_spmd

F32 = mybir.dt.float32
I8 = mybir.dt.int8
Q4 = 7
EPS = 1e-8

B, H, S, D = 2, 32, 2048, 128
N_CORES = 8
HEADS_PER_CORE = (B * H) // N_CORES  # 8

# Engine assignment for the dequant pass, cycled per tile.
PASS2_PATTERN = ("gpsimd", "scalar", "gpsimd", "scalar")


def _bcast(ap: bass.AP, d: int) -> bass.AP:
    """[128, j] AP -> [128, j, d] AP with step-0 innermost (broadcast)."""
    return bass.AP(ap.tensor, ap.offset, [ap.ap[0], [ap.ap[1][0], ap.ap[1][1]], [0, d]])


def build_nc(heads: int = HEADS_PER_CORE, seq: int = S):
    """Per-core Bass program: `heads` heads of all four slabs, emitting the
    seq-concatenated dequantized K/V."""
    j = seq // 128
    rows = heads * seq

    nc = bacc.Bacc(
        "TRN2",
        target_bir_lowering=False,
        debug=False,
        enable_asserts=True,
        num_devices=1,
    )

    ins = {
        name: nc.dram_tensor(name, [rows, D], F32, kind="ExternalInput")
        for name in ("k_cache", "k_new", "v_cache", "v_new")
    }
    k_out = nc.dram_tensor("k_out", [2 * rows, D], F32, kind="ExternalOutput")
    v_out = nc.dram_tensor("v_out", [2 * rows, D], F32, kind="ExternalOutput")

    in_views = {
        name: t.ap().rearrange("(h p j) d -> h p (j d)", h=heads, p=128)
        for name, t in ins.items()
    }
    out_views = {
        "k": k_out.ap().rearrange("(t p j) d -> t p (j d)", t=2 * heads, p=128),
        "v": v_out.ap().rearrange("(t p j) d -> t p (j d)", t=2 * heads, p=128),
    }

    slabs = [
        ("k_cache", "k", 0),
        ("k_new", "k", 1),
        ("v_cache", "v", 0),
        ("v_new", "v", 1),
    ]

    with tile.TileContext(nc) as tc:
        with (
            tc.tile_pool(name="xin", bufs=10) as xpool,
            tc.tile_pool(name="xi8", bufs=8) as qpool,
            tc.tile_pool(name="oout", bufs=8) as opool,
            tc.tile_pool(name="stats", bufs=12) as spool,
        ):
            tile_idx = 0
            for h in range(heads):
                for in_name, out_name, half in slabs:
                    x = xpool.tile([128, j * 128], F32, tag="x")
                    nc.sync.dma_start(x[:], in_views[in_name][h])
                    x3 = x[:].rearrange("p (jj d) -> p jj d", d=128)

                    am = spool.tile([128, j], F32, tag="am")
                    nc.vector.tensor_reduce(
                        am[:],
                        x3,
                        axis=mybir.AxisListType.X,
                        op=mybir.AluOpType.max,
                        apply_absolute_value=True,
                    )
                    s = spool.tile([128, j], F32, tag="s")
                    nc.vector.tensor_scalar(
                        s[:],
                        am[:],
                        1.0 / Q4,
                        EPS,
                        op0=mybir.AluOpType.mult,
                        op1=mybir.AluOpType.max,
                    )
                    inv = spool.tile([128, j], F32, tag="inv")
                    nc.vector.reciprocal(inv[:], s[:])

                    # pass1: xi = rne_int8(x * inv)
                    xi = qpool.tile([128, j * 128], I8, tag="xi")
                    xi3 = xi[:].rearrange("p (jj d) -> p jj d", d=128)
                    nc.vector.tensor_tensor(
                        xi3, x3, _bcast(inv[:], 128), op=mybir.AluOpType.mult
                    )

                    # pass2: out = xi * s
                    o = opool.tile([128, j * 128], F32, tag="o")
                    o3 = o[:].rearrange("p (jj d) -> p jj d", d=128)
                    n_tiles = heads * len(slabs)
                    if tile_idx >= n_tiles - 2:
                        # closing stretch: short chain on the lightly
                        # loaded gpsimd so the drain tail after the last
                        # input DMA is minimal
                        eng = "gpsimd"
                    else:
                        eng = PASS2_PATTERN[tile_idx % len(PASS2_PATTERN)]
                    if eng == "vector":
                        nc.vector.tensor_tensor(
                            o3, xi3, _bcast(s[:], 128), op=mybir.AluOpType.mult
                        )
                    elif eng == "gpsimd":
                        nc.gpsimd.tensor_tensor(
                            o3, xi3, _bcast(s[:], 128), op=mybir.AluOpType.mult
                        )
                    else:
                        for jj in range(j):
                            nc.scalar.activation(
                                o[:, jj * 128 : (jj + 1) * 128],
                                xi[:, jj * 128 : (jj + 1) * 128],
                                mybir.ActivationFunctionType.Copy,
                                bias=0.0,
                                scale=s[:, jj : jj + 1],
                            )

                    # Each output DMA issues from the engine that produced
                    # the tile, so no DMA issue ever waits on a foreign
                    # engine's sem (no head-of-line blocking on sync for
                    # inputs, none on scalar/gpsimd for outputs).
                    out_ap = out_views[out_name][h * 2 + half]
                    if eng == "gpsimd":
                        nc.gpsimd.dma_start(out_ap, o[:])
                    else:
                        nc.scalar.dma_start(out_ap, o[:])
                    tile_idx += 1

    nc.compile()
    return nc


_NC_CACHE: dict = {}

# Extra kwargs for run_bass_kernel_spmd (e.g. {"trace": True} from a test
# harness wanting an NTFF profile). Unused by the grading path.
RUN_KWARGS: dict = {}


def _get_nc():
    if "nc" not in _NC_CACHE:
        _NC_CACHE["nc"] = build_nc()
    return _NC_CACHE["nc"]


def kernel(k_cache, v_cache, k_new, v_new, _results_hook=None):
    nc = _get_nc()

    def shard(a):
        # [B, H, S, D] -> per-core [HEADS_PER_CORE * S, D]
        a = np.ascontiguousarray(a, dtype=np.float32).reshape(B * H, S, D)
        return [
            np.ascontiguousarray(
                a[c * HEADS_PER_CORE : (c + 1) * HEADS_PER_CORE].reshape(-1, D)
            )
            for c in range(N_CORES)
        ]

    shards = {
        name: shard(arr)
        for name, arr in (
            ("k_cache", k_cache),
            ("v_cache", v_cache),
            ("k_new", k_new),
            ("v_new", v_new),
        )
    }
    in_maps = [{name: shards[name][c] for name in shards} for c in range(N_CORES)]

    res = run_bass_kernel_spmd(
        nc, in_maps, core_ids=list(range(N_CORES)), **RUN_KWARGS
    )
    if _results_hook is not None:
        _results_hook(res)

    def gather(name):
        full = np.empty((B * H, 2 * S, D), np.float32)
        for c in range(N_CORES):
            full[c * HEADS_PER_CORE : (c + 1) * HEADS_PER_CORE] = res.results[c][
                name
            ].reshape(HEADS_PER_CORE, 2 * S, D)
        return full.reshape(B, H, 2 * S, D)

    return gather("k_out"), gather("v_out")
